# revision 1
# baseline (speedup 1.0000x reference)
"""Bass/Trainium2 kernel for nn_Attention (B=4, N=2048, IN=256, HID=1024,
D=1024, OUT=256, H=8 heads), SPMD over 8 NeuronCores.

Sharding: core c handles batch b = c//2 and head-group g = c%2 (4 heads,
512 of the 1024 inner features).  Layer-1 of each QKV MLP is recomputed on
both cores of a batch (cheap); the output projection is computed per
head-group and the two partial products are summed on the host (plus bias).

Mask compaction: ~half the tokens are masked out (key mask) and masked
queries only output the bias row.  The host applies ONE permutation
(valid tokens first) to q, k and v inputs, so the kernel runs on
NP = ceil(max_valid/128)*128 tokens instead of N=2048.  Using the same
permutation on both sides keeps the no-self-attention mask a true
diagonal.  Masked/padded key rows get an additive -30000 before exp.

Per-core dataflow (laid out so no on-chip transposes are ever needed):
  xT (256,NP) -> L1 feature-major h=(1024,NP) tanh -> L2:
     qT,kT feature-major bf16 (512,NP) = 4 head tiles [128,NP]
     v token-major bf16 (NP,512) (bias added via rank-1 matmul)
  attention per (head, q-chunk): S^T tiles [128 k-tok, qw] = kT_t.T @ qT;
     key mask enters as the per-partition bias of the Exp activation; the
     diagonal is one [128,128] additive DVE op; denominators via a
     [128,128] all-ones stationary matmul over the bf16 exp-accumulator
     (sum arrives broadcast across partitions); 1/s = exp(-ln s) on the
     Scalar engine; y^T accumulates in PSUM and is copied out immediately.
  proj: out^T = Wp_g^T @ (y^T * 1/s) in bf16.
"""

import numpy as np

B, N, IN_DIM, HID, D, OUT_DIM, H = 4, 2048, 256, 1024, 1024, 256, 8
NCORES = 8
HG = 2                 # head groups (cores per batch)
DG = D // HG           # 512 features per group
HEADS_G = H // HG      # 4 heads per core
Dh = D // H            # 128
NEG = -30000.0         # additive mask value (exp underflows to 0)

_CACHE = {}


def _chunks(total, size):
    out = []
    o = 0
    while o < total:
        s = min(size, total - o)
        out.append((o, s))
        o += s
    return out


def _build_nc(NP):
    import concourse.mybir as mybir
    import concourse.tile as tile
    from concourse import bacc
    from contextlib import ExitStack

    dt = mybir.dt
    f32 = dt.float32
    f32r = dt.float32r
    bf16 = dt.bfloat16
    AF = mybir.ActivationFunctionType
    ALU = mybir.AluOpType

    # Keep Ln and Exp in ONE activation-table set: blank out the funcs of
    # every exp/ln-capable set except exp_and_others (tanh+exp, phase A)
    # and natural_log_exp_and_others (exp+ln, phase B), so the table-load
    # pass never picks a set that would thrash between Ln and Exp.
    # Positions (= act_func_set_id) are preserved.
    if not getattr(bacc, "_act_tables_patched", False):
        from concourse import hw_specs as _hw
        _orig_get = _hw.get_activation_tables

        def _patched(arch):
            tables = dict(_orig_get(arch))
            AFT = mybir.ActivationFunctionType
            keep = {"exp_and_others", "natural_log_exp_and_others"}
            for name in tables:
                if name in keep:
                    continue
                fns = tables[name]
                if AFT.Exp in fns or AFT.Ln in fns:
                    tables[name] = set()
            return tables

        _patched.__wrapped__ = _orig_get
        bacc.get_activation_tables = _patched
        bacc._act_tables_patched = True

    nc = bacc.Bacc("TRN2", target_bir_lowering=False, debug=False)

    # ---- DRAM I/O ----
    xqT = nc.dram_tensor("xqT", [IN_DIM, NP], f32r, kind="ExternalInput")
    xkT = nc.dram_tensor("xkT", [IN_DIM, NP], f32r, kind="ExternalInput")
    xvT = nc.dram_tensor("xvT", [IN_DIM, NP], f32r, kind="ExternalInput")
    wq1 = nc.dram_tensor("wq1", [IN_DIM, HID], f32r, kind="ExternalInput")
    wk1 = nc.dram_tensor("wk1", [IN_DIM, HID], f32r, kind="ExternalInput")
    wv1 = nc.dram_tensor("wv1", [IN_DIM, HID], f32r, kind="ExternalInput")
    bq1 = nc.dram_tensor("bq1", [128, HID // 128], f32, kind="ExternalInput")
    bk1 = nc.dram_tensor("bk1", [128, HID // 128], f32, kind="ExternalInput")
    bv1 = nc.dram_tensor("bv1", [128, HID // 128], f32, kind="ExternalInput")
    wq2 = nc.dram_tensor("wq2", [HID, DG], bf16, kind="ExternalInput")
    wk2 = nc.dram_tensor("wk2", [HID, DG], bf16, kind="ExternalInput")
    wv2 = nc.dram_tensor("wv2", [HID, DG], bf16, kind="ExternalInput")
    bq2 = nc.dram_tensor("bq2", [128, DG // 128], f32, kind="ExternalInput")
    bk2 = nc.dram_tensor("bk2", [128, DG // 128], f32, kind="ExternalInput")
    bv2r = nc.dram_tensor("bv2r", [128, DG], f32r, kind="ExternalInput")
    onesc = nc.dram_tensor("onesc", [128, 128], bf16, kind="ExternalInput")
    e0Td = nc.dram_tensor("e0Td", [128, 128], f32r, kind="ExternalInput")
    wpb = nc.dram_tensor("wpb", [DG, OUT_DIM], bf16, kind="ExternalInput")
    kmadd = nc.dram_tensor("kmadd", [128, NP // 128], f32,
                           kind="ExternalInput")
    dneg = nc.dram_tensor("dneg", [128, 128], f32, kind="ExternalInput")
    outT = nc.dram_tensor("outT", [OUT_DIM, NP], f32, kind="ExternalOutput")

    KT1 = IN_DIM // 128          # 2  k-tiles in layer 1
    KT2 = HID // 128             # 8  k-tiles in layer 2
    MT1 = HID // 128             # 8  m-tiles in layer 1
    NTOK = NP // 128             # key-token tiles
    QCH = _chunks(NP, 1024)      # attention q-chunks
    THC = _chunks(NP, 1024)      # MLP token chunks

    with tile.TileContext(nc) as tc, ExitStack() as ctx:
        # pools (PSUM: ps 3x2 banks + psy 1x2 banks = 8 banks)
        ps = ctx.enter_context(tc.tile_pool(name="ps", bufs=3, space="PSUM"))
        psy = ctx.enter_context(tc.tile_pool(name="psy", bufs=1, space="PSUM"))
        singles = ctx.enter_context(tc.tile_pool(name="singles", bufs=1))
        xt_pool = ctx.enter_context(tc.tile_pool(name="xt", bufs=4))
        w1_pool = ctx.enter_context(tc.tile_pool(name="w1", bufs=4))
        w2_pool = ctx.enter_context(tc.tile_pool(name="w2", bufs=8))
        h_pool = ctx.enter_context(tc.tile_pool(name="h", bufs=8))
        qk_pool = ctx.enter_context(tc.tile_pool(name="qk", bufs=8))
        v_pool = ctx.enter_context(
            tc.tile_pool(name="v", bufs=(NTOK + 3) // 4))
        pt_pool = ctx.enter_context(tc.tile_pool(name="pt", bufs=6))
        sacc_pool = ctx.enter_context(tc.tile_pool(name="sacc", bufs=3))
        ysc_pool = ctx.enter_context(tc.tile_pool(name="ysc", bufs=6))
        rb_pool = ctx.enter_context(tc.tile_pool(name="rb", bufs=3))
        y2s_pool = ctx.enter_context(tc.tile_pool(name="y2s", bufs=3))
        out_pool = ctx.enter_context(tc.tile_pool(name="out", bufs=2))

        # constants
        ones128 = singles.tile([128, 128], bf16, tag="ones128")
        nc.sync.dma_start(out=ones128, in_=onesc[:, :])
        e0T = singles.tile([128, 128], f32r, tag="e0T")
        nc.sync.dma_start(out=e0T, in_=e0Td[:, :])
        km_sb = singles.tile([128, NP // 128], f32, tag="km")
        nc.sync.dma_start(out=km_sb, in_=kmadd[:, :])
        dneg_sb = singles.tile([128, 128], f32, tag="dneg")
        nc.sync.dma_start(out=dneg_sb, in_=dneg[:, :])
        bv2_sb = singles.tile([128, DG], f32r, tag="bv2")
        nc.sync.dma_start(out=bv2_sb, in_=bv2r[:, :])
        wp_sb = singles.tile([128, HEADS_G, OUT_DIM], bf16, tag="wp")
        nc.sync.dma_start(
            out=wp_sb, in_=wpb.rearrange("(h p) o -> p h o", p=128)
        )
        b1_sb = {}
        b2_sb = {}
        for t, (b1d, b2d) in {
            "q": (bq1, bq2), "k": (bk1, bk2), "v": (bv1, None)
        }.items():
            b1_sb[t] = singles.tile(
                [128, HID // 128], f32, tag=f"b1{t}", name=f"b1{t}")
            nc.sync.dma_start(out=b1_sb[t], in_=b1d[:, :])
            if b2d is not None:
                b2_sb[t] = singles.tile(
                    [128, DG // 128], f32, tag=f"b2{t}", name=f"b2{t}")
                nc.sync.dma_start(out=b2_sb[t], in_=b2d[:, :])

        # persistent activations
        qT = [qk_pool.tile([128, NP], bf16, tag="qk", name=f"qT{i}")
              for i in range(HEADS_G)]
        kT = [qk_pool.tile([128, NP], bf16, tag="qk", name=f"kT{i}")
              for i in range(HEADS_G)]
        v_sb = [v_pool.tile([128, 4 * DG], bf16, tag="v", name=f"v{i}")
                for i in range((NTOK + 3) // 4)]

        # ---------------- phase A: the three MLPs (v first: all Tanh
        # activations retire before attention's Exp stream begins) --------
        for t, xd, w1d, w2d in (
            ("v", xvT, wv1, wv2), ("k", xkT, wk1, wk2), ("q", xqT, wq1, wq2)
        ):
            w1_sb = []
            for k in range(KT1):
                w1t = w1_pool.tile([128, HID], f32r, tag="w1")
                nc.sync.dma_start(out=w1t, in_=w1d[k * 128:(k + 1) * 128, :])
                w1_sb.append(w1t)
            w2_sb = []
            for k in range(KT2):
                w2t = w2_pool.tile([128, DG], bf16, tag="w2")
                nc.sync.dma_start(out=w2t, in_=w2d[k * 128:(k + 1) * 128, :])
                w2_sb.append(w2t)

            for t0, tsz in THC:
                tok_sl = slice(t0, t0 + tsz)
                xts = []
                for k in range(KT1):
                    xt = xt_pool.tile([128, 1024], f32r, tag="xt")
                    nc.sync.dma_start(
                        out=xt[:, :tsz], in_=xd[k * 128:(k + 1) * 128, tok_sl]
                    )
                    xts.append(xt)
                # layer 1 (feature-major)
                h_sb = []
                for m in range(MT1):
                    p1 = ps.tile([128, 1024], f32, tag="ps")
                    for k in range(KT1):
                        for q0, qs in _chunks(tsz, 512):
                            nc.tensor.matmul(
                                p1[:, q0:q0 + qs],
                                w1_sb[k][:, m * 128:(m + 1) * 128],
                                xts[k][:, q0:q0 + qs],
                                start=(k == 0), stop=(k == KT1 - 1),
                            )
                    ht = h_pool.tile([128, 1024], bf16, tag="h")
                    nc.scalar.activation(
                        out=ht[:, :tsz], in_=p1[:, :tsz], func=AF.Tanh,
                        bias=b1_sb[t][:, m:m + 1], scale=1.0,
                    )
                    h_sb.append(ht)
                # layer 2
                if t in ("q", "k"):
                    dst = qT if t == "q" else kT
                    for m in range(DG // 128):       # head tiles
                        p2 = ps.tile([128, 1024], f32, tag="ps")
                        for k in range(KT2):
                            for q0, qs in _chunks(tsz, 512):
                                nc.tensor.matmul(
                                    p2[:, q0:q0 + qs],
                                    w2_sb[k][:, m * 128:(m + 1) * 128],
                                    h_sb[k][:, q0:q0 + qs],
                                    start=(k == 0), stop=(k == KT2 - 1),
                                )
                        nc.vector.tensor_scalar_add(
                            out=dst[m][:, tok_sl], in0=p2[:, :tsz],
                            scalar1=b2_sb[t][:, m:m + 1],
                        )
                else:
                    # v: token-major [tok, feat], bias via rank-1 matmul
                    ntiles = tsz // 128
                    for tp in range(0, ntiles, 2):   # pairs of token tiles
                        npair = min(2, ntiles - tp)
                        pv = ps.tile([128, 1024], f32, tag="ps")
                        for tt in range(npair):
                            sl = slice(tt * 512, (tt + 1) * 512)
                            for k in range(KT2):
                                nc.tensor.matmul(
                                    pv[:, sl],
                                    h_sb[k][:, (tp + tt) * 128:
                                            (tp + tt + 1) * 128],
                                    w2_sb[k][:, :],
                                    start=(k == 0), stop=False,
                                )
                            nc.tensor.matmul(
                                pv[:, sl], e0T[:, :], bv2_sb[:, :],
                                start=False, stop=True,
                            )
                        tok0 = t0 // 128 + tp
                        nc.vector.tensor_copy(
                            out=v_sb[tok0 // 4][
                                :, (tok0 % 4) * 512:
                                (tok0 % 4 + npair) * 512],
                            in_=pv[:, :npair * 512],
                        )

        # ---------------- phase B: attention + projection ----------------
        for q0, qw in QCH:
            ysc_tiles = []
            for hd in range(HEADS_G):
                y2 = psy.tile([128, 1024], f32, tag="y2")
                sacc = sacc_pool.tile([128, 1024], bf16, tag="sacc")
                for kt in range(NTOK):
                    st = ps.tile([128, 1024], f32, tag="ps")
                    for c0, cs in _chunks(qw, 512):
                        nc.tensor.matmul(
                            st[:, c0:c0 + cs],
                            kT[hd][:, kt * 128:(kt + 1) * 128],
                            qT[hd][:, q0 + c0:q0 + c0 + cs],
                            start=True, stop=True,
                        )
                    off = kt * 128 - q0
                    if 0 <= off <= qw - 128:
                        nc.vector.tensor_tensor(
                            st[:, off:off + 128], st[:, off:off + 128],
                            dneg_sb, ALU.add,
                        )
                    pt = pt_pool.tile([128, 1024], bf16, tag="pt")
                    nc.scalar.activation(
                        out=pt[:, :qw], in_=st[:, :qw], func=AF.Exp,
                        bias=km_sb[:, kt:kt + 1], scale=1.0,
                    )
                    if kt == 0:
                        nc.vector.tensor_copy(
                            out=sacc[:, :qw], in_=pt[:, :qw])
                    else:
                        nc.vector.tensor_tensor(
                            sacc[:, :qw], sacc[:, :qw], pt[:, :qw], ALU.add)
                    vt = v_sb[kt // 4][
                        :, (kt % 4) * 512 + hd * 128:
                        (kt % 4) * 512 + (hd + 1) * 128]
                    for c0, cs in _chunks(qw, 512):
                        nc.tensor.matmul(
                            y2[:, c0:c0 + cs], vt,
                            pt[:, c0:c0 + cs],
                            start=(kt == 0), stop=(kt == NTOK - 1),
                        )
                # free the y2 PSUM slot immediately
                y2s = y2s_pool.tile([128, 1024], f32, tag="y2s")
                nc.vector.tensor_copy(out=y2s[:, :qw], in_=y2[:, :qw])
                # denominators: all-ones stationary matmul -> sums broadcast
                aux = ps.tile([128, 1024], f32, tag="ps")
                for c0, cs in _chunks(qw, 512):
                    nc.tensor.matmul(
                        aux[:, c0:c0 + cs], ones128[:, :],
                        sacc[:, c0:c0 + cs],
                        start=True, stop=True,
                    )
                # 1/s = exp(-ln(s)) on the Scalar engine
                lns = rb_pool.tile([128, 1024], f32, tag="rb")
                nc.scalar.activation(
                    out=lns[:, :qw], in_=aux[:, :qw], func=AF.Ln, scale=1.0)
                rb2 = rb_pool.tile([128, 1024], f32, tag="rb")
                nc.scalar.activation(
                    out=rb2[:, :qw], in_=lns[:, :qw], func=AF.Exp,
                    scale=-1.0)
                ysc = ysc_pool.tile([128, 1024], bf16, tag="ysc")
                nc.vector.tensor_tensor(
                    ysc[:, :qw], y2s[:, :qw], rb2[:, :qw], ALU.mult)
                ysc_tiles.append(ysc)
            # projection for this q-chunk
            for od in range(OUT_DIM // 128):
                pp = ps.tile([128, 1024], f32, tag="ps")
                for c0, cs in _chunks(qw, 512):
                    for hd in range(HEADS_G):
                        nc.tensor.matmul(
                            pp[:, c0:c0 + cs],
                            wp_sb[:, hd, od * 128:(od + 1) * 128],
                            ysc_tiles[hd][:, c0:c0 + cs],
                            start=(hd == 0), stop=(hd == HEADS_G - 1),
                        )
                ot = out_pool.tile([128, 1024], f32, tag="out")
                nc.vector.tensor_copy(out=ot[:, :qw], in_=pp[:, :qw])
                nc.sync.dma_start(
                    out=outT[od * 128:(od + 1) * 128, q0:q0 + qw],
                    in_=ot[:, :qw],
                )

    nc.compile()
    return nc


def _row0_pad(row, nrows):
    out = np.zeros((nrows, row.shape[0]), np.float32)
    out[0] = row
    return out


def _e0t():
    out = np.zeros((128, 128), np.float32)
    out[0, :] = 1.0
    return out


def _perm_np(mask_b):
    """Valid-first stable permutation and valid count for one batch."""
    maskf = mask_b.astype(np.float32)
    perm = np.argsort(1.0 - maskf, kind="stable")
    nv = int(maskf.sum())
    return perm, nv


def _pad_tokens(x, NP):
    """x: (N, F) -> (NP, F) zero-padded/truncated token dim."""
    out = np.zeros((NP, x.shape[1]), np.float32)
    n = min(NP, x.shape[0])
    out[:n] = x[:n]
    return out


def _prep_core_inputs(inputs, b, g, NP):
    import ml_dtypes

    f32 = np.float32
    bf = ml_dtypes.bfloat16
    sl = slice(g * DG, (g + 1) * DG)
    scale = float(Dh) ** -0.5
    perm, nv = _perm_np(inputs["mask"][b, :, 0])
    km = np.full(NP, NEG, f32)
    km[:nv] = 0.0
    dn = np.zeros((128, 128), f32)
    np.fill_diagonal(dn, NEG)

    def ptok(x):   # permute tokens valid-first, pad to NP
        return _pad_tokens(x[perm].astype(f32), NP)

    return {
        "xqT": np.ascontiguousarray(ptok(inputs["query"][b]).T),
        "xkT": np.ascontiguousarray(ptok(inputs["key"][b]).T),
        "xvT": np.ascontiguousarray(ptok(inputs["value"][b]).T),
        "wq1": np.ascontiguousarray(inputs["Wq1"].astype(f32)),
        "wk1": np.ascontiguousarray(inputs["Wk1"].astype(f32)),
        "wv1": np.ascontiguousarray(inputs["Wv1"].astype(f32)),
        "bq1": np.ascontiguousarray(
            inputs["bq1"].astype(f32).reshape(HID // 128, 128).T),
        "bk1": np.ascontiguousarray(
            inputs["bk1"].astype(f32).reshape(HID // 128, 128).T),
        "bv1": np.ascontiguousarray(
            inputs["bv1"].astype(f32).reshape(HID // 128, 128).T),
        "wq2": np.ascontiguousarray(
            (inputs["Wq2"][:, sl].astype(f32) * scale).astype(bf)),
        "wk2": np.ascontiguousarray(inputs["Wk2"][:, sl].astype(bf)),
        "wv2": np.ascontiguousarray(inputs["Wv2"][:, sl].astype(bf)),
        "bq2": np.ascontiguousarray(
            (inputs["bq2"][sl].astype(f32) * scale).reshape(DG // 128, 128).T),
        "bk2": np.ascontiguousarray(
            inputs["bk2"][sl].astype(f32).reshape(DG // 128, 128).T),
        "bv2r": _row0_pad(inputs["bv2"][sl].astype(f32), 128),
        "onesc": np.ones((128, 128), bf),
        "e0Td": _e0t(),
        "wpb": np.ascontiguousarray(inputs["Wp"][sl, :].astype(bf)),
        "kmadd": np.ascontiguousarray(km.reshape(NP // 128, 128).T),
        "dneg": dn,
    }


def kernel(**inputs):
    import sys
    if "/opt/trn_rl_repo" not in sys.path:
        sys.path.insert(0, "/opt/trn_rl_repo")
    from concourse.bass_utils import run_bass_kernel_spmd

    inputs = {k: np.asarray(v) for k, v in inputs.items()}

    nv_max = int(inputs["mask"][:, :, 0].sum(axis=1).max())
    NP = ((nv_max + 127) // 128) * 128

    if _CACHE.get("NP") != NP:
        _CACHE["nc"] = _build_nc(NP)
        _CACHE["NP"] = NP
    nc = _CACHE["nc"]

    in_maps = [
        _prep_core_inputs(inputs, c // HG, c % HG, NP) for c in range(NCORES)
    ]

    res = run_bass_kernel_spmd(nc, in_maps, core_ids=list(range(NCORES)))
    results = res.results

    bp = inputs["bp"].astype(np.float32)
    out = np.empty((B, N, OUT_DIM), np.float32)
    for b in range(B):
        acc = results[b * HG]["outT"].astype(np.float32)
        for g in range(1, HG):
            acc = acc + results[b * HG + g]["outT"].astype(np.float32)
        perm, nv = _perm_np(inputs["mask"][b, :, 0])
        out[b] = bp[None, :]
        out[b, perm[:nv]] = acc.T[:nv] + bp[None, :]
    return out



# revision 2
# speedup vs baseline: 1.0908x; 1.0908x over previous
"""Bass/Trainium2 kernel for nn_Attention (B=4, N=2048, IN=256, HID=1024,
D=1024, OUT=256, H=8 heads), SPMD over 8 NeuronCores.

Sharding: core c handles batch b = c//2 and head-group g = c%2 (4 heads,
512 of the 1024 inner features).  Layer-1 of each QKV MLP is recomputed on
both cores of a batch (cheap); the output projection is computed per
head-group and the two partial products are summed on the host (plus bias).

Mask compaction: ~half the tokens are masked out (key mask) and masked
queries only output the bias row.  The host applies ONE permutation
(valid tokens first) to q, k and v inputs, so the kernel runs on
NP = ceil(max_valid/128)*128 tokens instead of N=2048.  Padded key rows
get an additive -30000 before exp (as the per-partition Exp bias).

All matmuls run in bf16 (x and W1 are converted on the host), full-width
NP tiles everywhere (no 128-wide tail instructions).

Per-core dataflow:
  phase A per type (v, k, q):  xT (256,NP) bf16 -> L1 h[m]=(128,NP) tanh
     bf16 (8 m-tiles) -> L2: qT,kT feature-major bf16 [128, 4, NP] (bias
     added on DVE); v token-major bf16 (bias via rank-1 e0 matmul).
  phase B per head, two-pass: S-pass: 9x { S^T tile [128 k-tok, NP] =
     kT_t.T @ qT (3 moving chunks); Exp with key-mask partition bias ->
     pt[kt] bf16; diagonal zeroed on GPSIMD (pt *= 1-I); denominator
     running-sum on DVE }.  Then y2-pass: 3 column sub-passes x 9
     accumulating AV matmuls into PSUM.  Denominators broadcast via
     all-ones stationary matmul; 1/s via DVE reciprocal_approx_fast;
     ysc = y2 * 1/s on DVE.  proj: out = sum_hd Wp_hd^T @ ysc_hd.

PSUM: "big" pool 2 x 3 banks (L1/L2 accumulators, S^T tiles, y2, aux),
"small" pool 2 x 1 bank (denominator chunks, projection tiles).
"""

import numpy as np

B, N, IN_DIM, HID, D, OUT_DIM, H = 4, 2048, 256, 1024, 1024, 256, 8
NCORES = 8
HG = 2                 # head groups (cores per batch)
DG = D // HG           # 512 features per group
HEADS_G = H // HG      # 4 heads per core
Dh = D // H            # 128
NEG = -30000.0         # additive mask value (exp underflows to 0)

_CACHE = {}


def _chunks(total, size):
    out = []
    o = 0
    while o < total:
        s = min(size, total - o)
        out.append((o, s))
        o += s
    return out


def _build_nc(NP):
    import concourse.mybir as mybir
    import concourse.tile as tile
    from concourse import bacc
    from contextlib import ExitStack

    dt = mybir.dt
    f32 = dt.float32
    bf16 = dt.bfloat16
    AF = mybir.ActivationFunctionType
    ALU = mybir.AluOpType

    # Keep all used activation funcs (Tanh, Exp) in ONE table set so the
    # table-load pass never thrashes.  Blank the funcs of every other
    # exp/ln-capable set; positions (= act_func_set_id) are preserved.
    if not getattr(bacc, "_act_tables_patched", False):
        from concourse import hw_specs as _hw
        _orig_get = _hw.get_activation_tables

        def _patched(arch):
            tables = dict(_orig_get(arch))
            AFT = mybir.ActivationFunctionType
            keep = {"exp_and_others", "natural_log_exp_and_others"}
            for name in tables:
                if name in keep:
                    continue
                fns = tables[name]
                if AFT.Exp in fns or AFT.Ln in fns:
                    tables[name] = set()
            return tables

        _patched.__wrapped__ = _orig_get
        bacc.get_activation_tables = _patched
        bacc._act_tables_patched = True

    nc = bacc.Bacc("TRN2", target_bir_lowering=False, debug=False)

    # ---- DRAM I/O ----
    xd_ = {}
    w1_ = {}
    w2_ = {}
    b1_ = {}
    for t in ("v", "k", "q"):
        xd_[t] = nc.dram_tensor(f"x{t}T", [IN_DIM, NP], bf16,
                                kind="ExternalInput")
        w1_[t] = nc.dram_tensor(f"w{t}1", [IN_DIM, HID], bf16,
                                kind="ExternalInput")
        w2_[t] = nc.dram_tensor(f"w{t}2", [HID, DG], bf16,
                                kind="ExternalInput")
        b1_[t] = nc.dram_tensor(f"b1{t}", [128, HID // 128], f32,
                                kind="ExternalInput")
    b2_ = {
        "q": nc.dram_tensor("b2q", [128, DG // 128], f32,
                            kind="ExternalInput"),
        "k": nc.dram_tensor("b2k", [128, DG // 128], f32,
                            kind="ExternalInput"),
    }
    bv2row = nc.dram_tensor("bv2row", [128, DG], bf16, kind="ExternalInput")
    e0d = nc.dram_tensor("e0d", [128, 128], bf16, kind="ExternalInput")
    onesd = nc.dram_tensor("onesd", [128, 128], bf16, kind="ExternalInput")
    eyeCd = nc.dram_tensor("eyeCd", [128, 128], bf16, kind="ExternalInput")
    kmd = nc.dram_tensor("kmd", [128, NP // 128], f32, kind="ExternalInput")
    wpb = nc.dram_tensor("wpb", [DG, OUT_DIM], bf16, kind="ExternalInput")
    outT = nc.dram_tensor("outT", [OUT_DIM, NP], f32, kind="ExternalOutput")

    KT1 = IN_DIM // 128          # 2  k-tiles in layer 1
    KT2 = HID // 128             # 8  k-tiles in layer 2
    MT1 = HID // 128             # 8  m-tiles in layer 1
    NTOK = NP // 128             # key-token tiles
    C3 = _chunks(NP, 512)        # moving-dim chunks (bank-aligned)
    NPB = ((NP + 511) // 512) * 512   # psum cols rounded to full banks

    with tile.TileContext(nc) as tc, ExitStack() as ctx:
        # PSUM: big = 2 x 3 banks, small = 2 x 1 bank  (8 banks total)
        big = ctx.enter_context(tc.tile_pool(name="big", bufs=2,
                                             space="PSUM"))
        small = ctx.enter_context(tc.tile_pool(name="small", bufs=2,
                                               space="PSUM"))
        singles = ctx.enter_context(tc.tile_pool(name="singles", bufs=1))
        xt_pool = ctx.enter_context(tc.tile_pool(name="xt", bufs=4))
        w1_pool = ctx.enter_context(tc.tile_pool(name="w1", bufs=4))
        w2_pool = ctx.enter_context(tc.tile_pool(name="w2", bufs=16))
        h_pool = ctx.enter_context(tc.tile_pool(name="h", bufs=8))
        qk_pool = ctx.enter_context(tc.tile_pool(name="qk", bufs=2))
        v_pool = ctx.enter_context(
            tc.tile_pool(name="v", bufs=(NTOK + 3) // 4))
        pt_pool = ctx.enter_context(tc.tile_pool(name="pt", bufs=13))
        sacc_pool = ctx.enter_context(tc.tile_pool(name="sacc", bufs=2))
        rb_pool = ctx.enter_context(tc.tile_pool(name="rb", bufs=2))
        ysc_pool = ctx.enter_context(tc.tile_pool(name="ysc", bufs=5))
        out_pool = ctx.enter_context(tc.tile_pool(name="out", bufs=4))

        # ---- constants (small, loaded first) ----
        b1_sb = {}
        for t in ("v", "k", "q"):
            b1_sb[t] = singles.tile(
                [128, HID // 128], f32, tag=f"b1{t}", name=f"b1{t}")
            nc.sync.dma_start(out=b1_sb[t], in_=b1_[t][:, :])
        b2_sb = {}
        for t in ("q", "k"):
            b2_sb[t] = singles.tile(
                [128, DG // 128], f32, tag=f"b2{t}", name=f"b2{t}")
            nc.sync.dma_start(out=b2_sb[t], in_=b2_[t][:, :])
        bv2_sb = singles.tile([128, DG], bf16, tag="bv2")
        nc.sync.dma_start(out=bv2_sb, in_=bv2row[:, :])
        e0_sb = singles.tile([128, 128], bf16, tag="e0")
        nc.sync.dma_start(out=e0_sb, in_=e0d[:, :])

        # persistent activations
        qT = qk_pool.tile([128, HEADS_G, NP], bf16, tag="qk", name="qT")
        kT = qk_pool.tile([128, HEADS_G, NP], bf16, tag="qk", name="kT")
        v_sb = [v_pool.tile([128, 4 * DG], bf16, tag="v", name=f"v{i}")
                for i in range((NTOK + 3) // 4)]

        # ---------------- phase A: the three MLPs --------------------
        for t in ("v", "k", "q"):
            w1_sb = []
            for k in range(KT1):
                w1t = w1_pool.tile([128, HID], bf16, tag="w1")
                nc.sync.dma_start(
                    out=w1t, in_=w1_[t][k * 128:(k + 1) * 128, :])
                w1_sb.append(w1t)
            xts = []
            for k in range(KT1):
                xt = xt_pool.tile([128, NP], bf16, tag="xt")
                nc.sync.dma_start(
                    out=xt, in_=xd_[t][k * 128:(k + 1) * 128, :])
                xts.append(xt)
            w2_sb = []
            for k in range(KT2):
                w2t = w2_pool.tile([128, DG], bf16, tag="w2")
                nc.sync.dma_start(
                    out=w2t, in_=w2_[t][k * 128:(k + 1) * 128, :])
                w2_sb.append(w2t)
            if t == "q":
                # phase-B constants: needed only after q's MLP
                ones_sb = singles.tile([128, 128], bf16, tag="ones")
                nc.sync.dma_start(out=ones_sb, in_=onesd[:, :])
                eyeC_sb = singles.tile([128, 128], bf16, tag="eyeC")
                nc.sync.dma_start(out=eyeC_sb, in_=eyeCd[:, :])
                km_sb = singles.tile([128, NP // 128], f32, tag="km")
                nc.sync.dma_start(out=km_sb, in_=kmd[:, :])
                wp_sb = singles.tile([128, HEADS_G, OUT_DIM], bf16, tag="wp")
                nc.sync.dma_start(
                    out=wp_sb, in_=wpb.rearrange("(h p) o -> p h o", p=128))

            # layer 1 (feature-major, full NP width)
            h_sb = []
            for m in range(MT1):
                p1 = big.tile([128, NPB], f32, tag="big")
                for k in range(KT1):
                    for c0, cs in C3:
                        nc.tensor.matmul(
                            p1[:, c0:c0 + cs],
                            w1_sb[k][:, m * 128:(m + 1) * 128],
                            xts[k][:, c0:c0 + cs],
                            start=(k == 0), stop=(k == KT1 - 1),
                        )
                ht = h_pool.tile([128, NP], bf16, tag="h")
                nc.scalar.activation(
                    out=ht, in_=p1[:, :NP], func=AF.Tanh,
                    bias=b1_sb[t][:, m:m + 1], scale=1.0,
                )
                h_sb.append(ht)
            # layer 2
            if t in ("q", "k"):
                dst = qT if t == "q" else kT
                for m in range(DG // 128):       # head tiles
                    p2 = big.tile([128, NPB], f32, tag="big")
                    for k in range(KT2):
                        for c0, cs in C3:
                            nc.tensor.matmul(
                                p2[:, c0:c0 + cs],
                                w2_sb[k][:, m * 128:(m + 1) * 128],
                                h_sb[k][:, c0:c0 + cs],
                                start=(k == 0), stop=(k == KT2 - 1),
                            )
                    nc.vector.tensor_scalar_add(
                        out=dst[:, m, :], in0=p2[:, :NP],
                        scalar1=b2_sb[t][:, m:m + 1],
                    )
            else:
                # v: token-major [tok, feat], bias via rank-1 matmul
                for tp in range(0, NTOK, 2):     # pairs of token tiles
                    npair = min(2, NTOK - tp)
                    pv = big.tile([128, NPB], f32, tag="big")
                    for tt in range(npair):
                        sl = slice(tt * DG, (tt + 1) * DG)
                        for k in range(KT2):
                            nc.tensor.matmul(
                                pv[:, sl],
                                h_sb[k][:, (tp + tt) * 128:
                                        (tp + tt + 1) * 128],
                                w2_sb[k][:, :],
                                start=(k == 0), stop=False,
                            )
                        nc.tensor.matmul(
                            pv[:, sl], e0_sb[:, :], bv2_sb[:, :],
                            start=False, stop=True,
                        )
                    nc.vector.tensor_copy(
                        out=v_sb[tp // 4][
                            :, (tp % 4) * DG:(tp % 4 + npair) * DG],
                        in_=pv[:, :npair * DG],
                    )

        # ---------------- phase B: attention + projection ----------------
        ysc_tiles = []
        for hd in range(HEADS_G):
            # --- S-pass: scores + exp + diag-zero + denominator sums ---
            pts = []
            sacc = sacc_pool.tile([128, NP], bf16, tag="sacc")
            for kt in range(NTOK):
                st = big.tile([128, NPB], f32, tag="big")
                for c0, cs in C3:
                    nc.tensor.matmul(
                        st[:, c0:c0 + cs],
                        kT[:, hd, kt * 128:(kt + 1) * 128],
                        qT[:, hd, c0:c0 + cs],
                        start=True, stop=True,
                    )
                pt = pt_pool.tile([128, NP], bf16, tag="pt")
                nc.scalar.activation(
                    out=pt, in_=st[:, :NP], func=AF.Exp,
                    bias=km_sb[:, kt:kt + 1], scale=1.0,
                )
                # no self-attention: zero the diagonal block on GPSIMD
                db = kt * 128
                nc.gpsimd.tensor_tensor(
                    pt[:, db:db + 128], pt[:, db:db + 128], eyeC_sb,
                    ALU.mult,
                )
                if kt == 0:
                    nc.vector.tensor_copy(out=sacc, in_=pt)
                else:
                    nc.vector.tensor_tensor(sacc, sacc, pt, ALU.add)
                pts.append(pt)
            # --- y2-pass: 3 column sub-passes of accumulating AV matmuls ---
            y2 = big.tile([128, NPB], f32, tag="big")
            for c0, cs in C3:
                for kt in range(NTOK):
                    vt = v_sb[kt // 4][
                        :, (kt % 4) * DG + hd * 128:
                        (kt % 4) * DG + (hd + 1) * 128]
                    nc.tensor.matmul(
                        y2[:, c0:c0 + cs], vt,
                        pts[kt][:, c0:c0 + cs],
                        start=(kt == 0), stop=(kt == NTOK - 1),
                    )
            # --- denominators: all-ones stationary matmul -> broadcast ---
            rb = rb_pool.tile([128, NP], f32, tag="rb")
            for c0, cs in C3:
                aux = small.tile([128, 512], f32, tag="small")
                nc.tensor.matmul(
                    aux[:, :cs], ones_sb[:, :], sacc[:, c0:c0 + cs],
                    start=True, stop=True,
                )
                nc.vector.reciprocal_approx_fast(
                    out=rb[:, c0:c0 + cs], in_=aux[:, :cs])
            ysc = ysc_pool.tile([128, NP], bf16, tag="ysc")
            nc.vector.tensor_tensor(ysc, y2[:, :NP], rb, ALU.mult)
            ysc_tiles.append(ysc)
        # --- projection ---
        for od in range(OUT_DIM // 128):
            for c0, cs in C3:
                pp = small.tile([128, 512], f32, tag="small")
                for hd in range(HEADS_G):
                    nc.tensor.matmul(
                        pp[:, :cs],
                        wp_sb[:, hd, od * 128:(od + 1) * 128],
                        ysc_tiles[hd][:, c0:c0 + cs],
                        start=(hd == 0), stop=(hd == HEADS_G - 1),
                    )
                ot = out_pool.tile([128, 512], f32, tag="out")
                if od == 0:
                    nc.vector.tensor_copy(out=ot[:, :cs], in_=pp[:, :cs])
                else:
                    nc.scalar.activation(
                        out=ot[:, :cs], in_=pp[:, :cs], func=AF.Copy,
                        scale=1.0)
                nc.sync.dma_start(
                    out=outT[od * 128:(od + 1) * 128, c0:c0 + cs],
                    in_=ot[:, :cs],
                )

    nc.compile()
    return nc


def _perm_np(mask_b):
    """Valid-first stable permutation and valid count for one batch."""
    maskf = mask_b.astype(np.float32)
    perm = np.argsort(1.0 - maskf, kind="stable")
    nv = int(maskf.sum())
    return perm, nv


def _pad_tokens(x, NP):
    """x: (N, F) -> (NP, F) zero-padded/truncated token dim."""
    out = np.zeros((NP, x.shape[1]), np.float32)
    n = min(NP, x.shape[0])
    out[:n] = x[:n]
    return out


def _prep_core_inputs(inputs, b, g, NP):
    import ml_dtypes

    f32 = np.float32
    bf = ml_dtypes.bfloat16
    sl = slice(g * DG, (g + 1) * DG)
    scale = float(Dh) ** -0.5
    perm, nv = _perm_np(inputs["mask"][b, :, 0])
    km = np.full(NP, NEG, f32)
    km[:nv] = 0.0
    e0 = np.zeros((128, 128), f32)
    e0[0, :] = 1.0
    eyeC = np.ones((128, 128), f32) - np.eye(128, dtype=f32)
    bv2r = np.zeros((128, DG), f32)
    bv2r[0] = inputs["bv2"][sl].astype(f32)

    def ptok(x):   # permute tokens valid-first, pad to NP
        return _pad_tokens(x[perm].astype(f32), NP)

    return {
        "xqT": np.ascontiguousarray(ptok(inputs["query"][b]).T).astype(bf),
        "xkT": np.ascontiguousarray(ptok(inputs["key"][b]).T).astype(bf),
        "xvT": np.ascontiguousarray(ptok(inputs["value"][b]).T).astype(bf),
        "wq1": np.ascontiguousarray(inputs["Wq1"].astype(bf)),
        "wk1": np.ascontiguousarray(inputs["Wk1"].astype(bf)),
        "wv1": np.ascontiguousarray(inputs["Wv1"].astype(bf)),
        "b1q": np.ascontiguousarray(
            inputs["bq1"].astype(f32).reshape(HID // 128, 128).T),
        "b1k": np.ascontiguousarray(
            inputs["bk1"].astype(f32).reshape(HID // 128, 128).T),
        "b1v": np.ascontiguousarray(
            inputs["bv1"].astype(f32).reshape(HID // 128, 128).T),
        "wq2": np.ascontiguousarray(
            (inputs["Wq2"][:, sl].astype(f32) * scale).astype(bf)),
        "wk2": np.ascontiguousarray(inputs["Wk2"][:, sl].astype(bf)),
        "wv2": np.ascontiguousarray(inputs["Wv2"][:, sl].astype(bf)),
        "b2q": np.ascontiguousarray(
            (inputs["bq2"][sl].astype(f32) * scale).reshape(DG // 128, 128).T),
        "b2k": np.ascontiguousarray(
            inputs["bk2"][sl].astype(f32).reshape(DG // 128, 128).T),
        "bv2row": bv2r.astype(bf),
        "e0d": e0.astype(bf),
        "onesd": np.ones((128, 128), bf),
        "eyeCd": eyeC.astype(bf),
        "kmd": np.ascontiguousarray(km.reshape(NP // 128, 128).T),
        "wpb": np.ascontiguousarray(inputs["Wp"][sl, :].astype(bf)),
    }


def kernel(**inputs):
    import sys
    if "/opt/trn_rl_repo" not in sys.path:
        sys.path.insert(0, "/opt/trn_rl_repo")
    from concourse.bass_utils import run_bass_kernel_spmd

    inputs = {k: np.asarray(v) for k, v in inputs.items()}

    nv_max = int(inputs["mask"][:, :, 0].sum(axis=1).max())
    NP = ((nv_max + 127) // 128) * 128

    if _CACHE.get("NP") != NP:
        _CACHE["nc"] = _build_nc(NP)
        _CACHE["NP"] = NP
    nc = _CACHE["nc"]

    in_maps = [
        _prep_core_inputs(inputs, c // HG, c % HG, NP) for c in range(NCORES)
    ]

    res = run_bass_kernel_spmd(nc, in_maps, core_ids=list(range(NCORES)))
    results = res.results

    bp = inputs["bp"].astype(np.float32)
    out = np.empty((B, N, OUT_DIM), np.float32)
    for b in range(B):
        acc = results[b * HG]["outT"].astype(np.float32)
        for g in range(1, HG):
            acc = acc + results[b * HG + g]["outT"].astype(np.float32)
        perm, nv = _perm_np(inputs["mask"][b, :, 0])
        out[b] = bp[None, :]
        out[b, perm[:nv]] = acc.T[:nv] + bp[None, :]
    return out


# revision 8
# speedup vs baseline: 1.2024x; 1.1023x over previous
"""Bass/Trainium2 kernel for nn_Attention (B=4, N=2048, IN=256, HID=1024,
D=1024, OUT=256, H=8 heads), SPMD over 8 NeuronCores.

Sharding: core c handles batch b = c//2 and head-group g = c%2 (4 heads,
512 of the 1024 inner features).  Layer-1 of each QKV MLP is recomputed on
both cores of a batch (cheap); the output projection is computed per
head-group and the two partial products are summed on the host (plus bias).

Mask compaction: ~half the tokens are masked out (key mask) and masked
queries only output the bias row.  The host applies ONE permutation
(valid tokens first) to q, k and v inputs, so the kernel runs on
NP = ceil(max_valid/128)*128 tokens instead of N=2048.  Padded key rows
get an additive -30000 before exp (as the per-partition Exp bias).

All matmuls run in bf16; full-width NP tiles everywhere.  DMA triggers
cost ~0.6us each on their issuing queue, so inputs are fetched as ONE
merged tile per tensor and triggers are spread across the sync, gpsimd
and vector queues.  A short garbage-data matmul warmup at t~6us ramps the
PE p-state (full clock needs ~3us of continuous execution) while the
first input tiles stream in.

Schedule (the Exp stream on the Scalar engine is the phase-B limiter, so
three heads' score/exp work runs inside the v-MLP window where Scalar is
otherwise idle):
  1. k-MLP, q-MLP, v-L1   (tensor-bound, Scalar does tanh only)
  2. v-L2 token-pairs interleaved with S-units of heads 0..2
     (S-unit kt: S^T tile [128,NP] = kT_kt.T @ qT via 3 chunk matmuls;
      Exp with key-mask partition bias -> pt bf16; diagonal zeroed on
      GPSIMD (pt *= 1-I); denominator running-sum on DVE; at kt==8:
      per-chunk all-ones stationary matmul -> broadcast sums -> DVE
      reciprocal_approx_fast -> rb[hd])
  3. S-units of head 3 interleaved with the 9 y2-groups of heads 0..2
     (y2-group (hd,c): 9 accumulating AV matmuls into a 1-bank PSUM
      chunk, then ysc[hd][:,c] = y2c * rb[hd][:,c] on DVE), then
     y2-groups of head 3, projection, output copies + DMA.

PSUM: "big" pool 2 x 3 banks (L1/L2 accumulators and S^T tiles), "small"
pool 2 x 1 bank (warmup, denominator chunks, y2 chunks, projection).
"""

import numpy as np

B, N, IN_DIM, HID, D, OUT_DIM, H = 4, 2048, 256, 1024, 1024, 256, 8
NCORES = 8
HG = 2                 # head groups (cores per batch)
DG = D // HG           # 512 features per group
HEADS_G = H // HG      # 4 heads per core
Dh = D // H            # 128
NEG = -30000.0         # additive mask value (exp underflows to 0)

_CACHE = {}


def _chunks(total, size):
    out = []
    o = 0
    while o < total:
        s = min(size, total - o)
        out.append((o, s))
        o += s
    return out


def _build_nc(NP):
    import concourse.mybir as mybir
    import concourse.tile as tile
    from concourse import bacc
    from contextlib import ExitStack

    dt = mybir.dt
    f32 = dt.float32
    bf16 = dt.bfloat16
    AF = mybir.ActivationFunctionType
    ALU = mybir.AluOpType

    # Keep all used activation funcs (Tanh, Exp) in ONE table set so the
    # table-load pass never thrashes.
    if not getattr(bacc, "_act_tables_patched", False):
        from concourse import hw_specs as _hw
        _orig_get = _hw.get_activation_tables

        def _patched(arch):
            tables = dict(_orig_get(arch))
            AFT = mybir.ActivationFunctionType
            keep = {"exp_and_others", "natural_log_exp_and_others"}
            for name in tables:
                if name in keep:
                    continue
                fns = tables[name]
                if AFT.Exp in fns or AFT.Ln in fns:
                    tables[name] = set()
            return tables

        _patched.__wrapped__ = _orig_get
        bacc.get_activation_tables = _patched
        bacc._act_tables_patched = True

    nc = bacc.Bacc("TRN2", target_bir_lowering=False, debug=False)

    # ---- DRAM I/O ----
    xd_ = {}
    w1_ = {}
    w2_ = {}
    for t in ("k", "q", "v"):
        xd_[t] = nc.dram_tensor(f"x{t}T", [IN_DIM, NP], bf16,
                                kind="ExternalInput")
        w1_[t] = nc.dram_tensor(f"w{t}1", [IN_DIM, HID], bf16,
                                kind="ExternalInput")
        w2_[t] = nc.dram_tensor(f"w{t}2", [HID, DG], bf16,
                                kind="ExternalInput")
    bpk = nc.dram_tensor("bpk", [128, 32], f32, kind="ExternalInput")
    bv2row = nc.dram_tensor("bv2row", [128, DG], bf16, kind="ExternalInput")
    e0d = nc.dram_tensor("e0d", [128, 128], bf16, kind="ExternalInput")
    onesd = nc.dram_tensor("onesd", [128, 128], bf16, kind="ExternalInput")
    eyeCd = nc.dram_tensor("eyeCd", [128, 128], bf16, kind="ExternalInput")
    kmd = nc.dram_tensor("kmd", [128, NP // 128], f32, kind="ExternalInput")
    wpb = nc.dram_tensor("wpb", [DG, OUT_DIM], bf16, kind="ExternalInput")
    outT = nc.dram_tensor("outT", [OUT_DIM, NP], f32, kind="ExternalOutput")

    KT1 = IN_DIM // 128          # 2  k-tiles in layer 1
    KT2 = HID // 128             # 8  k-tiles in layer 2
    MT1 = HID // 128             # 8  m-tiles in layer 1
    NTOK = NP // 128             # key-token tiles
    C3 = _chunks(NP, 512)        # moving-dim chunks (bank-aligned)
    NPB = ((NP + 511) // 512) * 512   # psum cols rounded to full banks
    # bias-pack column offsets: b1 per type (8 each), then b2q, b2k (4 each)
    B1OFF = {"v": 0, "k": 8, "q": 16}
    B2OFF = {"q": 24, "k": 28}

    with tile.TileContext(nc) as tc, ExitStack() as ctx:
        # PSUM: big = 2 x 3 banks, small = 2 x 1 bank  (8 banks total)
        big = ctx.enter_context(tc.tile_pool(name="big", bufs=2,
                                             space="PSUM"))
        small = ctx.enter_context(tc.tile_pool(name="small", bufs=2,
                                               space="PSUM"))
        singles = ctx.enter_context(tc.tile_pool(name="singles", bufs=1))
        xt_pool = ctx.enter_context(tc.tile_pool(name="xt", bufs=2))
        w1_pool = ctx.enter_context(tc.tile_pool(name="w1", bufs=2))
        w2_pool = ctx.enter_context(tc.tile_pool(name="w2", bufs=2))
        h_pool = ctx.enter_context(tc.tile_pool(name="h", bufs=8))
        qk_pool = ctx.enter_context(tc.tile_pool(name="qk", bufs=2))
        v_pool = ctx.enter_context(
            tc.tile_pool(name="v", bufs=(NTOK + 3) // 4))
        pt_pool = ctx.enter_context(tc.tile_pool(name="pt", bufs=28))
        sacc_pool = ctx.enter_context(tc.tile_pool(name="sacc", bufs=2))
        rb_pool = ctx.enter_context(tc.tile_pool(name="rb", bufs=4))
        ysc_pool = ctx.enter_context(tc.tile_pool(name="ysc", bufs=4))
        out_pool = ctx.enter_context(tc.tile_pool(name="out", bufs=4))

        # ---- warmup: ramp the PE p-state on zeroed garbage data (the PE
        # needs ~3us of continuous execution to reach full clock; any idle
        # gap drops it back for the next ~3us, so the schedule below is
        # built to keep the tensor queue gap-free) ----
        wu = singles.tile([128, 512], bf16, tag="wu")
        nc.gpsimd.memset(wu[:, :], 0)
        wups = small.tile([128, 512], f32, tag="small")
        for _ in range(8):
            nc.tensor.matmul(wups[:, :], wu[:, :128], wu[:, :],
                             start=True, stop=True)
        nc.vector.tensor_copy(out=wu[:, 0:1], in_=wups[:, 0:1])

        # ---- small constants on the scalar queue (idle at start); w2
        # weights also go there per-type so the sync queue streams only
        # the critical w1/x tiles in first-use order ----
        bpk_sb = singles.tile([128, 32], f32, tag="bpk")
        nc.scalar.dma_start(out=bpk_sb, in_=bpk[:, :])
        ones_sb = singles.tile([128, 128], bf16, tag="ones")
        nc.scalar.dma_start(out=ones_sb, in_=onesd[:, :])
        eyeC_sb = singles.tile([128, 128], bf16, tag="eyeC")
        nc.scalar.dma_start(out=eyeC_sb, in_=eyeCd[:, :])
        km_sb = singles.tile([128, NP // 128], f32, tag="km")
        nc.scalar.dma_start(out=km_sb, in_=kmd[:, :])
        wp_sb = singles.tile([128, HEADS_G, OUT_DIM], bf16, tag="wp")
        nc.scalar.dma_start(
            out=wp_sb, in_=wpb.rearrange("(h p) o -> p h o", p=128))
        bv2_sb = singles.tile([128, DG], bf16, tag="bv2")
        nc.gpsimd.dma_start(out=bv2_sb, in_=bv2row[:, :])
        e0_sb = singles.tile([128, 128], bf16, tag="e0")
        nc.gpsimd.dma_start(out=e0_sb, in_=e0d[:, :])

        # persistent activations
        qT = qk_pool.tile([128, HEADS_G, NP], bf16, tag="qk", name="qT")
        kT = qk_pool.tile([128, HEADS_G, NP], bf16, tag="qk", name="kT")
        v_sb = [v_pool.tile([128, 4 * DG], bf16, tag="v", name=f"v{i}")
                for i in range((NTOK + 3) // 4)]

        # ---------------- phase 1: k-MLP, q-MLP, v-L1 --------------------
        h_v = None
        w2v = None
        for t in ("k", "q", "v"):
            w1t = w1_pool.tile([128, KT1, HID], bf16, tag="w1")
            nc.sync.dma_start(
                out=w1t, in_=w1_[t].rearrange("(k p) h -> p k h", p=128))
            xt = xt_pool.tile([128, KT1, NP], bf16, tag="xt")
            nc.sync.dma_start(
                out=xt, in_=xd_[t].rearrange("(k p) n -> p k n", p=128))
            w2t = w2_pool.tile([128, KT2, DG], bf16, tag="w2")
            nc.scalar.dma_start(
                out=w2t, in_=w2_[t].rearrange("(k p) d -> p k d", p=128))
            # layer 1 (feature-major, full NP width)
            h_sb = []
            for m in range(MT1):
                p1 = big.tile([128, NPB], f32, tag="big")
                for k in range(KT1):
                    for c0, cs in C3:
                        nc.tensor.matmul(
                            p1[:, c0:c0 + cs],
                            w1t[:, k, m * 128:(m + 1) * 128],
                            xt[:, k, c0:c0 + cs],
                            start=(k == 0), stop=(k == KT1 - 1),
                        )
                ht = h_pool.tile([128, NP], bf16, tag="h")
                nc.scalar.activation(
                    out=ht, in_=p1[:, :NP], func=AF.Tanh,
                    bias=bpk_sb[:, B1OFF[t] + m:B1OFF[t] + m + 1], scale=1.0,
                )
                h_sb.append(ht)
            if t == "v":
                h_v = h_sb
                w2v = w2t
                break            # v-L2 is interleaved into phase 2
            # layer 2 for q/k (feature-major)
            dst = qT if t == "q" else kT
            for m in range(DG // 128):       # head tiles
                p2 = big.tile([128, NPB], f32, tag="big")
                for k in range(KT2):
                    for c0, cs in C3:
                        nc.tensor.matmul(
                            p2[:, c0:c0 + cs],
                            w2t[:, k, m * 128:(m + 1) * 128],
                            h_sb[k][:, c0:c0 + cs],
                            start=(k == 0), stop=(k == KT2 - 1),
                        )
                nc.vector.tensor_scalar_add(
                    out=dst[:, m, :], in0=p2[:, :NP],
                    scalar1=bpk_sb[:, B2OFF[t] + m:B2OFF[t] + m + 1],
                )

        # ---------------- phase 2/3 helpers ------------------------------
        pts = {}
        rb = {}
        sacc_cur = [None]

        def emit_s_unit(hd, kt):
            st = big.tile([128, NPB], f32, tag="big")
            for c0, cs in C3:
                nc.tensor.matmul(
                    st[:, c0:c0 + cs],
                    kT[:, hd, kt * 128:(kt + 1) * 128],
                    qT[:, hd, c0:c0 + cs],
                    start=True, stop=True,
                )
            pt = pt_pool.tile([128, NP], bf16, tag="pt")
            nc.scalar.activation(
                out=pt, in_=st[:, :NP], func=AF.Exp,
                bias=km_sb[:, kt:kt + 1], scale=1.0,
            )
            # no self-attention: zero the diagonal block on GPSIMD
            db = kt * 128
            nc.gpsimd.tensor_tensor(
                pt[:, db:db + 128], pt[:, db:db + 128], eyeC_sb, ALU.mult)
            if kt == 0:
                sacc_cur[0] = sacc_pool.tile([128, NP], bf16, tag="sacc",
                                             name="sacc")
                nc.vector.tensor_copy(out=sacc_cur[0], in_=pt)
            else:
                nc.vector.tensor_tensor(sacc_cur[0], sacc_cur[0], pt,
                                        ALU.add)
            pts[(hd, kt)] = pt
            if kt == NTOK - 1:
                # denominators -> broadcast sums -> 1/s
                rbt = rb_pool.tile([128, NP], f32, tag="rb")
                for c0, cs in C3:
                    aux = small.tile([128, 512], f32, tag="small")
                    nc.tensor.matmul(
                        aux[:, :cs], ones_sb[:, :],
                        sacc_cur[0][:, c0:c0 + cs],
                        start=True, stop=True,
                    )
                    nc.vector.reciprocal_approx_fast(
                        out=rbt[:, c0:c0 + cs], in_=aux[:, :cs])
                rb[hd] = rbt

        ysc = [ysc_pool.tile([128, NP], bf16, tag="ysc", name=f"ysc{i}")
               for i in range(HEADS_G)]

        def emit_y2_group(hd, c0, cs):
            y2c = small.tile([128, 512], f32, tag="small")
            for kt in range(NTOK):
                vt = v_sb[kt // 4][
                    :, (kt % 4) * DG + hd * 128:
                    (kt % 4) * DG + (hd + 1) * 128]
                nc.tensor.matmul(
                    y2c[:, :cs], vt, pts[(hd, kt)][:, c0:c0 + cs],
                    start=(kt == 0), stop=(kt == NTOK - 1),
                )
            nc.vector.tensor_tensor(
                ysc[hd][:, c0:c0 + cs], y2c[:, :cs], rb[hd][:, c0:c0 + cs],
                ALU.mult)

        # ---------------- phase 2: v-L2 + S-units of heads 0..2 ----------
        # The v-L2 matmul stream (9 token tiles x (8 accum + 1 bias) = 81
        # matmuls) is interleaved 3-per-S-unit so the tensor queue always
        # has exp-independent work between the exp-gated S^T tiles.
        pv_cur = [None]

        def emit_v_mm(idx):
            tt, j = idx // (KT2 + 1), idx % (KT2 + 1)
            if j == 0:
                pv_cur[0] = small.tile([128, 512], f32, tag="small",
                                       name="pv")
            if j < KT2:
                nc.tensor.matmul(
                    pv_cur[0][:, :],
                    h_v[j][:, tt * 128:(tt + 1) * 128],
                    w2v[:, j, :],
                    start=(j == 0), stop=False,
                )
            else:
                nc.tensor.matmul(
                    pv_cur[0][:, :], e0_sb[:, :], bv2_sb[:, :],
                    start=False, stop=True,
                )
                nc.vector.tensor_copy(
                    out=v_sb[tt // 4][:, (tt % 4) * DG:(tt % 4 + 1) * DG],
                    in_=pv_cur[0][:, :],
                )

        s_units = [(hd, kt) for hd in range(HEADS_G - 1)
                   for kt in range(NTOK)]
        nvm = NTOK * (KT2 + 1)
        vi = 0
        for i, (hd, kt) in enumerate(s_units):
            emit_s_unit(hd, kt)
            vt_end = min(nvm, (nvm * (i + 1) + len(s_units) - 1)
                         // len(s_units))
            while vi < vt_end:
                emit_v_mm(vi)
                vi += 1
        while vi < nvm:
            emit_v_mm(vi)
            vi += 1

        # ---------------- phase 3: S(h3) + y2 groups + projection --------
        groups = [(hd, c0, cs) for hd in range(HEADS_G - 1)
                  for c0, cs in C3]
        for kt in range(NTOK):
            emit_s_unit(HEADS_G - 1, kt)
            if kt < len(groups):
                emit_y2_group(*groups[kt])
        for gi in range(NTOK, len(groups)):
            emit_y2_group(*groups[gi])
        for c0, cs in C3:
            emit_y2_group(HEADS_G - 1, c0, cs)
        # projection
        for od in range(OUT_DIM // 128):
            for c0, cs in C3:
                pp = small.tile([128, 512], f32, tag="small")
                for hd in range(HEADS_G):
                    nc.tensor.matmul(
                        pp[:, :cs],
                        wp_sb[:, hd, od * 128:(od + 1) * 128],
                        ysc[hd][:, c0:c0 + cs],
                        start=(hd == 0), stop=(hd == HEADS_G - 1),
                    )
                ot = out_pool.tile([128, 512], f32, tag="out")
                nc.scalar.activation(
                    out=ot[:, :cs], in_=pp[:, :cs], func=AF.Copy, scale=1.0)
                eng = nc.sync if od == 0 else nc.gpsimd
                eng.dma_start(
                    out=outT[od * 128:(od + 1) * 128, c0:c0 + cs],
                    in_=ot[:, :cs],
                )

    nc.compile()
    return nc


def _perm_np(mask_b):
    """Valid-first stable permutation and valid count for one batch."""
    maskf = mask_b.astype(np.float32)
    perm = np.argsort(1.0 - maskf, kind="stable")
    nv = int(maskf.sum())
    return perm, nv


def _pad_tokens(x, NP):
    """x: (N, F) -> (NP, F) zero-padded/truncated token dim."""
    out = np.zeros((NP, x.shape[1]), np.float32)
    n = min(NP, x.shape[0])
    out[:n] = x[:n]
    return out


def _prep_core_inputs(inputs, b, g, NP):
    import ml_dtypes

    f32 = np.float32
    bf = ml_dtypes.bfloat16
    sl = slice(g * DG, (g + 1) * DG)
    scale = float(Dh) ** -0.5
    perm, nv = _perm_np(inputs["mask"][b, :, 0])
    km = np.full(NP, NEG, f32)
    km[:nv] = 0.0
    e0 = np.zeros((128, 128), f32)
    e0[0, :] = 1.0
    eyeC = np.ones((128, 128), f32) - np.eye(128, dtype=f32)
    bv2r = np.zeros((128, DG), f32)
    bv2r[0] = inputs["bv2"][sl].astype(f32)
    # bias pack: [b1v | b1k | b1q | b2q | b2k]  (cols 0:8, 8:16, 16:24,
    # 24:28, 28:32); b1 columns are the per-m-tile partition biases.
    bpk = np.zeros((128, 32), f32)
    bpk[:, 0:8] = inputs["bv1"].astype(f32).reshape(HID // 128, 128).T
    bpk[:, 8:16] = inputs["bk1"].astype(f32).reshape(HID // 128, 128).T
    bpk[:, 16:24] = inputs["bq1"].astype(f32).reshape(HID // 128, 128).T
    bpk[:, 24:28] = (inputs["bq2"][sl].astype(f32) * scale).reshape(
        DG // 128, 128).T
    bpk[:, 28:32] = inputs["bk2"][sl].astype(f32).reshape(DG // 128, 128).T

    def ptok(x):   # permute tokens valid-first, pad to NP
        return _pad_tokens(x[perm].astype(f32), NP)

    return {
        "xqT": np.ascontiguousarray(ptok(inputs["query"][b]).T).astype(bf),
        "xkT": np.ascontiguousarray(ptok(inputs["key"][b]).T).astype(bf),
        "xvT": np.ascontiguousarray(ptok(inputs["value"][b]).T).astype(bf),
        "wq1": np.ascontiguousarray(inputs["Wq1"].astype(bf)),
        "wk1": np.ascontiguousarray(inputs["Wk1"].astype(bf)),
        "wv1": np.ascontiguousarray(inputs["Wv1"].astype(bf)),
        "wq2": np.ascontiguousarray(
            (inputs["Wq2"][:, sl].astype(f32) * scale).astype(bf)),
        "wk2": np.ascontiguousarray(inputs["Wk2"][:, sl].astype(bf)),
        "wv2": np.ascontiguousarray(inputs["Wv2"][:, sl].astype(bf)),
        "bpk": bpk,
        "bv2row": bv2r.astype(bf),
        "e0d": e0.astype(bf),
        "onesd": np.ones((128, 128), bf),
        "eyeCd": eyeC.astype(bf),
        "kmd": np.ascontiguousarray(km.reshape(NP // 128, 128).T),
        "wpb": np.ascontiguousarray(inputs["Wp"][sl, :].astype(bf)),
    }


def kernel(**inputs):
    import sys
    if "/opt/trn_rl_repo" not in sys.path:
        sys.path.insert(0, "/opt/trn_rl_repo")
    from concourse.bass_utils import run_bass_kernel_spmd

    inputs = {k: np.asarray(v) for k, v in inputs.items()}

    nv_max = int(inputs["mask"][:, :, 0].sum(axis=1).max())
    NP = ((nv_max + 127) // 128) * 128

    if _CACHE.get("NP") != NP:
        _CACHE["nc"] = _build_nc(NP)
        _CACHE["NP"] = NP
    nc = _CACHE["nc"]

    in_maps = [
        _prep_core_inputs(inputs, c // HG, c % HG, NP) for c in range(NCORES)
    ]

    res = run_bass_kernel_spmd(nc, in_maps, core_ids=list(range(NCORES)))
    results = res.results

    bp = inputs["bp"].astype(np.float32)
    out = np.empty((B, N, OUT_DIM), np.float32)
    for b in range(B):
        acc = results[b * HG]["outT"].astype(np.float32)
        for g in range(1, HG):
            acc = acc + results[b * HG + g]["outT"].astype(np.float32)
        perm, nv = _perm_np(inputs["mask"][b, :, 0])
        out[b] = bp[None, :]
        out[b, perm[:nv]] = acc.T[:nv] + bp[None, :]
    return out


# revision 15
# speedup vs baseline: 1.2654x; 1.0524x over previous
"""Bass/Trainium2 kernel for nn_Attention (B=4, N=2048, IN=256, HID=1024,
D=1024, OUT=256, H=8 heads), SPMD over 8 NeuronCores.

Sharding: core c handles batch b = c//2 and head-group g = c%2 (4 heads,
512 of the 1024 inner features).  Layer-1 of each QKV MLP is recomputed on
both cores of a batch (cheap); the output projection is computed per
head-group and the two partial products are summed on the host (plus bias).

Mask compaction: ~half the tokens are masked out (key mask) and masked
queries only output the bias row.  The host applies ONE permutation
(valid tokens first) to q, k and v inputs, so the kernel runs on
NP = ceil(max_valid/128)*128 tokens instead of N=2048.  Padded key rows
get an additive -30000 before exp (as the per-partition Exp bias).

All matmuls run in bf16; full-width NP tiles everywhere.  DMA triggers
cost ~0.6us each on their issuing queue, so inputs are fetched as ONE
merged tile per tensor and triggers are spread across the sync, gpsimd
and vector queues.  A short garbage-data matmul warmup at t~6us ramps the
PE p-state (full clock needs ~3us of continuous execution) while the
first input tiles stream in.

Schedule (the Exp stream on the Scalar engine is the phase-B limiter, so
three heads' score/exp work runs inside the v-MLP window where Scalar is
otherwise idle):
  1. k-MLP, q-MLP, v-L1   (tensor-bound, Scalar does tanh only)
  2. v-L2 token-pairs interleaved with S-units of heads 0..2
     (S-unit kt: S^T tile [128,NP] = kT_kt.T @ qT via 3 chunk matmuls;
      Exp with key-mask partition bias -> pt bf16; diagonal zeroed on
      GPSIMD (pt *= 1-I); denominator running-sum on DVE; at kt==8:
      per-chunk all-ones stationary matmul -> broadcast sums -> DVE
      reciprocal_approx_fast -> rb[hd])
  3. S-units of head 3 interleaved with the 9 y2-groups of heads 0..2
     (y2-group (hd,c): 9 accumulating AV matmuls into a 1-bank PSUM
      chunk, then ysc[hd][:,c] = y2c * rb[hd][:,c] on DVE), then
     y2-groups of head 3, projection, output copies + DMA.

PSUM: "big" pool 2 x 3 banks (L1/L2 accumulators and S^T tiles), "small"
pool 2 x 1 bank (warmup, denominator chunks, y2 chunks, projection).
"""

import numpy as np

B, N, IN_DIM, HID, D, OUT_DIM, H = 4, 2048, 256, 1024, 1024, 256, 8
NCORES = 8
HG = 2                 # head groups (cores per batch)
DG = D // HG           # 512 features per group
HEADS_G = H // HG      # 4 heads per core
Dh = D // H            # 128
NEG = -30000.0         # additive mask value (exp underflows to 0)

_CACHE = {}


def _chunks(total, size):
    out = []
    o = 0
    while o < total:
        s = min(size, total - o)
        out.append((o, s))
        o += s
    return out


def _build_nc(NP):
    import concourse.mybir as mybir
    import concourse.tile as tile
    from concourse import bacc
    from contextlib import ExitStack

    dt = mybir.dt
    f32 = dt.float32
    bf16 = dt.bfloat16
    AF = mybir.ActivationFunctionType
    ALU = mybir.AluOpType

    # Keep all used activation funcs (Tanh, Exp) in ONE table set so the
    # table-load pass never thrashes.
    if not getattr(bacc, "_act_tables_patched", False):
        from concourse import hw_specs as _hw
        _orig_get = _hw.get_activation_tables

        def _patched(arch):
            tables = dict(_orig_get(arch))
            AFT = mybir.ActivationFunctionType
            keep = {"exp_and_others", "natural_log_exp_and_others"}
            for name in tables:
                if name in keep:
                    continue
                fns = tables[name]
                if AFT.Exp in fns or AFT.Ln in fns:
                    tables[name] = set()
            return tables

        _patched.__wrapped__ = _orig_get
        bacc.get_activation_tables = _patched
        bacc._act_tables_patched = True

    nc = bacc.Bacc("TRN2", target_bir_lowering=False, debug=False)

    # ---- DRAM I/O ----
    xd_ = {}
    w1_ = {}
    w2_ = {}
    for t in ("k", "q", "v"):
        xd_[t] = nc.dram_tensor(f"x{t}T", [IN_DIM, NP], bf16,
                                kind="ExternalInput")
        w1_[t] = nc.dram_tensor(f"w{t}1", [IN_DIM, HID], bf16,
                                kind="ExternalInput")
        w2_[t] = nc.dram_tensor(f"w{t}2", [HID, DG], bf16,
                                kind="ExternalInput")
    bpk = nc.dram_tensor("bpk", [128, 32], f32, kind="ExternalInput")
    bv2row = nc.dram_tensor("bv2row", [128, DG], bf16, kind="ExternalInput")
    e0d = nc.dram_tensor("e0d", [128, 128], bf16, kind="ExternalInput")
    onesd = nc.dram_tensor("onesd", [128, 128], bf16, kind="ExternalInput")
    eyeCd = nc.dram_tensor("eyeCd", [128, 128], bf16, kind="ExternalInput")
    kmd = nc.dram_tensor("kmd", [128, NP // 128], f32, kind="ExternalInput")
    wpb = nc.dram_tensor("wpb", [DG, OUT_DIM], bf16, kind="ExternalInput")
    outT = nc.dram_tensor("outT", [OUT_DIM, NP], bf16, kind="ExternalOutput")

    KT1 = IN_DIM // 128          # 2  k-tiles in layer 1
    KT2 = HID // 128             # 8  k-tiles in layer 2
    MT1 = HID // 128             # 8  m-tiles in layer 1
    NTOK = NP // 128             # key-token tiles
    C3 = _chunks(NP, 512)        # moving-dim chunks (bank-aligned)
    NPB = ((NP + 511) // 512) * 512   # psum cols rounded to full banks
    # bias-pack column offsets: b1 per type (8 each), then b2q, b2k (4 each)
    B1OFF = {"v": 0, "k": 8, "q": 16}
    B2OFF = {"q": 24, "k": 28}

    with tile.TileContext(nc) as tc, ExitStack() as ctx:
        # PSUM: big = 2 x 3 banks, small = 2 x 1 bank  (8 banks total)
        big = ctx.enter_context(tc.tile_pool(name="big", bufs=2,
                                             space="PSUM"))
        small = ctx.enter_context(tc.tile_pool(name="small", bufs=2,
                                               space="PSUM"))
        singles = ctx.enter_context(tc.tile_pool(name="singles", bufs=1))
        xt_pool = ctx.enter_context(tc.tile_pool(name="xt", bufs=2))
        w1_pool = ctx.enter_context(tc.tile_pool(name="w1", bufs=2))
        w2_pool = ctx.enter_context(tc.tile_pool(name="w2", bufs=2))
        h_pool = ctx.enter_context(tc.tile_pool(name="h", bufs=8))
        qk_pool = ctx.enter_context(tc.tile_pool(name="qk", bufs=2))
        v_pool = ctx.enter_context(
            tc.tile_pool(name="v", bufs=(NTOK + 3) // 4))
        pt_pool = ctx.enter_context(tc.tile_pool(name="pt", bufs=28))
        sacc_pool = ctx.enter_context(tc.tile_pool(name="sacc", bufs=2))
        rb_pool = ctx.enter_context(tc.tile_pool(name="rb", bufs=4))
        ysc_pool = ctx.enter_context(tc.tile_pool(name="ysc", bufs=4))
        out_pool = ctx.enter_context(tc.tile_pool(name="out", bufs=4))

        # ---- warmup: ramp the PE p-state on zeroed garbage data (the PE
        # needs ~3us of continuous execution to reach full clock; any idle
        # gap drops it back for the next ~3us, so the schedule below is
        # built to keep the tensor queue gap-free) ----
        wu = singles.tile([128, 512], bf16, tag="wu")
        nc.gpsimd.memset(wu[:, :], 0)
        wups = small.tile([128, 512], f32, tag="small")
        for _ in range(8):
            nc.tensor.matmul(wups[:, :], wu[:, :128], wu[:, :],
                             start=True, stop=True)
        nc.vector.tensor_copy(out=wu[:, 0:1], in_=wups[:, 0:1])

        # ---- small constants on the scalar queue (idle at start); w2
        # weights also go there per-type so the sync queue streams only
        # the critical w1/x tiles in first-use order ----
        bpk_sb = singles.tile([128, 32], f32, tag="bpk")
        nc.scalar.dma_start(out=bpk_sb, in_=bpk[:, :])
        ones_sb = singles.tile([128, 128], bf16, tag="ones")
        nc.scalar.dma_start(out=ones_sb, in_=onesd[:, :])
        eyeC_sb = singles.tile([128, 128], bf16, tag="eyeC")
        nc.scalar.dma_start(out=eyeC_sb, in_=eyeCd[:, :])
        km_sb = singles.tile([128, NP // 128], f32, tag="km")
        nc.scalar.dma_start(out=km_sb, in_=kmd[:, :])
        wp_sb = singles.tile([128, HEADS_G, OUT_DIM], bf16, tag="wp")
        nc.scalar.dma_start(
            out=wp_sb, in_=wpb.rearrange("(h p) o -> p h o", p=128))
        bv2_sb = singles.tile([128, DG], bf16, tag="bv2")
        nc.gpsimd.dma_start(out=bv2_sb, in_=bv2row[:, :])
        e0_sb = singles.tile([128, 128], bf16, tag="e0")
        nc.gpsimd.dma_start(out=e0_sb, in_=e0d[:, :])

        # persistent activations
        qT = qk_pool.tile([128, HEADS_G, NP], bf16, tag="qk", name="qT")
        kT = qk_pool.tile([128, HEADS_G, NP], bf16, tag="qk", name="kT")
        v_sb = [v_pool.tile([128, 4 * DG], bf16, tag="v", name=f"v{i}")
                for i in range((NTOK + 3) // 4)]

        # ---------------- phase 1: k-MLP, q-MLP, v-L1 --------------------
        h_v = None
        w2v = None
        for t in ("k", "q", "v"):
            w1t = w1_pool.tile([128, KT1, HID], bf16, tag="w1")
            w1ap = w1_[t].rearrange("(k p) h -> p k h", p=128)
            xt = xt_pool.tile([128, KT1, NP], bf16, tag="xt")
            xap = xd_[t].rearrange("(k p) n -> p k n", p=128)
            if t == "k":
                # split the first tiles so compute can start after ~half
                # the bytes have landed (first-use order on the sync queue)
                nc.sync.dma_start(out=w1t[:, :, :HID // 2],
                                  in_=w1ap[:, :, :HID // 2])
                nc.sync.dma_start(out=xt[:, 0, :], in_=xap[:, 0, :])
                nc.sync.dma_start(out=xt[:, 1, :], in_=xap[:, 1, :])
                nc.sync.dma_start(out=w1t[:, :, HID // 2:],
                                  in_=w1ap[:, :, HID // 2:])
            else:
                nc.sync.dma_start(out=w1t, in_=w1ap)
                nc.sync.dma_start(out=xt, in_=xap)
            w2t = w2_pool.tile([128, KT2, DG], bf16, tag="w2")
            nc.scalar.dma_start(
                out=w2t, in_=w2_[t].rearrange("(k p) d -> p k d", p=128))
            # layer 1 (feature-major, full NP width)
            h_sb = []
            for m in range(MT1):
                p1 = big.tile([128, NPB], f32, tag="big")
                for k in range(KT1):
                    for c0, cs in C3:
                        nc.tensor.matmul(
                            p1[:, c0:c0 + cs],
                            w1t[:, k, m * 128:(m + 1) * 128],
                            xt[:, k, c0:c0 + cs],
                            start=(k == 0), stop=(k == KT1 - 1),
                        )
                ht = h_pool.tile([128, NP], bf16, tag="h")
                nc.scalar.activation(
                    out=ht, in_=p1[:, :NP], func=AF.Tanh,
                    bias=bpk_sb[:, B1OFF[t] + m:B1OFF[t] + m + 1], scale=1.0,
                )
                h_sb.append(ht)
            if t == "v":
                h_v = h_sb
                w2v = w2t
                break            # v-L2 is interleaved into phase 2
            # layer 2 for q/k (feature-major)
            dst = qT if t == "q" else kT
            for m in range(DG // 128):       # head tiles
                p2 = big.tile([128, NPB], f32, tag="big")
                for k in range(KT2):
                    for c0, cs in C3:
                        nc.tensor.matmul(
                            p2[:, c0:c0 + cs],
                            w2t[:, k, m * 128:(m + 1) * 128],
                            h_sb[k][:, c0:c0 + cs],
                            start=(k == 0), stop=(k == KT2 - 1),
                        )
                nc.scalar.activation(
                    out=dst[:, m, :], in_=p2[:, :NP], func=AF.Identity,
                    bias=bpk_sb[:, B2OFF[t] + m:B2OFF[t] + m + 1], scale=1.0,
                )

        # ---------------- phase 2/3 helpers ------------------------------
        pts = {}
        rb = {}
        saccs = {}
        sacc_cur = [None]

        def emit_s_unit(hd, kt):
            st = big.tile([128, NPB], f32, tag="big")
            for c0, cs in C3:
                nc.tensor.matmul(
                    st[:, c0:c0 + cs],
                    kT[:, hd, kt * 128:(kt + 1) * 128],
                    qT[:, hd, c0:c0 + cs],
                    start=True, stop=True,
                )
            pt = pt_pool.tile([128, NP], bf16, tag="pt")
            nc.scalar.activation(
                out=pt, in_=st[:, :NP], func=AF.Exp,
                bias=km_sb[:, kt:kt + 1], scale=1.0,
            )
            # no self-attention: zero the diagonal block on GPSIMD
            db = kt * 128
            nc.gpsimd.tensor_tensor(
                pt[:, db:db + 128], pt[:, db:db + 128], eyeC_sb, ALU.mult)
            if kt == 0:
                sacc_cur[0] = sacc_pool.tile([128, NP], bf16, tag="sacc",
                                             name="sacc")
                nc.vector.tensor_copy(out=sacc_cur[0], in_=pt)
            else:
                nc.vector.tensor_tensor(sacc_cur[0], sacc_cur[0], pt,
                                        ALU.add)
            pts[(hd, kt)] = pt
            if kt == NTOK - 1:
                saccs[hd] = sacc_cur[0]

        def emit_aux(hd):
            # denominators -> broadcast sums -> 1/s.  Deferred a couple of
            # S-units past the head's last exp so the tensor queue never
            # stalls waiting for the DVE running-sum chain to finish.
            rbt = rb_pool.tile([128, NP], f32, tag="rb")
            for c0, cs in C3:
                aux = small.tile([128, 512], f32, tag="small")
                nc.tensor.matmul(
                    aux[:, :cs], ones_sb[:, :], saccs[hd][:, c0:c0 + cs],
                    start=True, stop=True,
                )
                nc.vector.reciprocal_approx_fast(
                    out=rbt[:, c0:c0 + cs], in_=aux[:, :cs])
            rb[hd] = rbt

        ysc = [ysc_pool.tile([128, NP], bf16, tag="ysc", name=f"ysc{i}")
               for i in range(HEADS_G)]

        def emit_y2_group(hd, c0, cs):
            y2c = small.tile([128, 512], f32, tag="small")
            for kt in range(NTOK):
                vt = v_sb[kt // 4][
                    :, (kt % 4) * DG + hd * 128:
                    (kt % 4) * DG + (hd + 1) * 128]
                nc.tensor.matmul(
                    y2c[:, :cs], vt, pts[(hd, kt)][:, c0:c0 + cs],
                    start=(kt == 0), stop=(kt == NTOK - 1),
                )
            nc.vector.tensor_tensor(
                ysc[hd][:, c0:c0 + cs], y2c[:, :cs], rb[hd][:, c0:c0 + cs],
                ALU.mult)

        # ---------------- phase 2: v-L2 + S-units of heads 0..2 ----------
        # The v-L2 matmul stream (9 token tiles x (8 accum + 1 bias) = 81
        # matmuls) is interleaved 3-per-S-unit so the tensor queue always
        # has exp-independent work between the exp-gated S^T tiles.
        pv_cur = [None]

        def emit_v_mm(idx):
            tt, j = idx // (KT2 + 1), idx % (KT2 + 1)
            if j == 0:
                pv_cur[0] = small.tile([128, 512], f32, tag="small",
                                       name="pv")
            if j < KT2:
                nc.tensor.matmul(
                    pv_cur[0][:, :],
                    h_v[j][:, tt * 128:(tt + 1) * 128],
                    w2v[:, j, :],
                    start=(j == 0), stop=False,
                )
            else:
                nc.tensor.matmul(
                    pv_cur[0][:, :], e0_sb[:, :], bv2_sb[:, :],
                    start=False, stop=True,
                )
                nc.vector.tensor_copy(
                    out=v_sb[tt // 4][:, (tt % 4) * DG:(tt % 4 + 1) * DG],
                    in_=pv_cur[0][:, :],
                )

        s_units = [(hd, kt) for hd in range(HEADS_G - 1)
                   for kt in range(NTOK)]
        nvm = NTOK * (KT2 + 1)
        vi = 0
        pend_aux = []
        for i, (hd, kt) in enumerate(s_units):
            emit_s_unit(hd, kt)
            if kt == NTOK - 1:
                pend_aux.append((hd, i))
            if pend_aux and i >= pend_aux[0][1] + 2:
                emit_aux(pend_aux.pop(0)[0])
            vt_end = min(nvm, (nvm * (i + 1) + len(s_units) - 1)
                         // len(s_units))
            while vi < vt_end:
                emit_v_mm(vi)
                vi += 1
        while vi < nvm:
            emit_v_mm(vi)
            vi += 1

        # ---------------- phase 3: S(h3) + y2 groups + projection --------
        dma_eng = [nc.sync, nc.gpsimd]

        def emit_proj(od, c0, cs):
            pp = small.tile([128, 512], f32, tag="small", name="pp")
            for hd in range(HEADS_G):
                nc.tensor.matmul(
                    pp[:, :cs],
                    wp_sb[:, hd, od * 128:(od + 1) * 128],
                    ysc[hd][:, c0:c0 + cs],
                    start=(hd == 0), stop=(hd == HEADS_G - 1),
                )
            ot = out_pool.tile([128, 512], bf16, tag="out", name="ot")
            nc.scalar.activation(
                out=ot[:, :cs], in_=pp[:, :cs], func=AF.Copy, scale=1.0)
            dma_eng[od].dma_start(
                out=outT[od * 128:(od + 1) * 128, c0:c0 + cs],
                in_=ot[:, :cs],
            )

        groups = [(hd, c0, cs) for hd in range(HEADS_G - 1)
                  for c0, cs in C3]
        h3 = HEADS_G - 1
        for kt in range(NTOK):
            emit_s_unit(h3, kt)
            if kt < len(groups):
                emit_y2_group(*groups[kt])
            if kt == 1 and pend_aux:
                emit_aux(pend_aux.pop(0)[0])
        for gi in range(NTOK, len(groups)):
            emit_y2_group(*groups[gi])
        # h3 denominators, then y2(h3) column groups with the projection
        # matmuls (and their output DMAs) interleaved right behind them
        emit_aux(h3)
        emit_y2_group(h3, *C3[0])
        for ci in range(1, len(C3)):
            emit_y2_group(h3, *C3[ci])
            emit_proj(0, *C3[ci - 1])
            emit_proj(1, *C3[ci - 1])
        emit_proj(0, *C3[-1])
        emit_proj(1, *C3[-1])

    nc.compile()
    return nc


def _perm_np(mask_b):
    """Valid-first stable permutation and valid count for one batch."""
    maskf = mask_b.astype(np.float32)
    perm = np.argsort(1.0 - maskf, kind="stable")
    nv = int(maskf.sum())
    return perm, nv


def _pad_tokens(x, NP):
    """x: (N, F) -> (NP, F) zero-padded/truncated token dim."""
    out = np.zeros((NP, x.shape[1]), np.float32)
    n = min(NP, x.shape[0])
    out[:n] = x[:n]
    return out


def _prep_core_inputs(inputs, b, g, NP):
    import ml_dtypes

    f32 = np.float32
    bf = ml_dtypes.bfloat16
    sl = slice(g * DG, (g + 1) * DG)
    scale = float(Dh) ** -0.5
    perm, nv = _perm_np(inputs["mask"][b, :, 0])
    km = np.full(NP, NEG, f32)
    km[:nv] = 0.0
    e0 = np.zeros((128, 128), f32)
    e0[0, :] = 1.0
    eyeC = np.ones((128, 128), f32) - np.eye(128, dtype=f32)
    bv2r = np.zeros((128, DG), f32)
    bv2r[0] = inputs["bv2"][sl].astype(f32)
    # bias pack: [b1v | b1k | b1q | b2q | b2k]  (cols 0:8, 8:16, 16:24,
    # 24:28, 28:32); b1 columns are the per-m-tile partition biases.
    bpk = np.zeros((128, 32), f32)
    bpk[:, 0:8] = inputs["bv1"].astype(f32).reshape(HID // 128, 128).T
    bpk[:, 8:16] = inputs["bk1"].astype(f32).reshape(HID // 128, 128).T
    bpk[:, 16:24] = inputs["bq1"].astype(f32).reshape(HID // 128, 128).T
    bpk[:, 24:28] = (inputs["bq2"][sl].astype(f32) * scale).reshape(
        DG // 128, 128).T
    bpk[:, 28:32] = inputs["bk2"][sl].astype(f32).reshape(DG // 128, 128).T

    def ptok(x):   # permute tokens valid-first, pad to NP
        return _pad_tokens(x[perm].astype(f32), NP)

    return {
        "xqT": np.ascontiguousarray(ptok(inputs["query"][b]).T).astype(bf),
        "xkT": np.ascontiguousarray(ptok(inputs["key"][b]).T).astype(bf),
        "xvT": np.ascontiguousarray(ptok(inputs["value"][b]).T).astype(bf),
        "wq1": np.ascontiguousarray(inputs["Wq1"].astype(bf)),
        "wk1": np.ascontiguousarray(inputs["Wk1"].astype(bf)),
        "wv1": np.ascontiguousarray(inputs["Wv1"].astype(bf)),
        "wq2": np.ascontiguousarray(
            (inputs["Wq2"][:, sl].astype(f32) * scale).astype(bf)),
        "wk2": np.ascontiguousarray(inputs["Wk2"][:, sl].astype(bf)),
        "wv2": np.ascontiguousarray(inputs["Wv2"][:, sl].astype(bf)),
        "bpk": bpk,
        "bv2row": bv2r.astype(bf),
        "e0d": e0.astype(bf),
        "onesd": np.ones((128, 128), bf),
        "eyeCd": eyeC.astype(bf),
        "kmd": np.ascontiguousarray(km.reshape(NP // 128, 128).T),
        "wpb": np.ascontiguousarray(inputs["Wp"][sl, :].astype(bf)),
    }


def kernel(**inputs):
    import sys
    if "/opt/trn_rl_repo" not in sys.path:
        sys.path.insert(0, "/opt/trn_rl_repo")
    from concourse.bass_utils import run_bass_kernel_spmd

    inputs = {k: np.asarray(v) for k, v in inputs.items()}

    nv_max = int(inputs["mask"][:, :, 0].sum(axis=1).max())
    NP = ((nv_max + 127) // 128) * 128

    if _CACHE.get("NP") != NP:
        _CACHE["nc"] = _build_nc(NP)
        _CACHE["NP"] = NP
    nc = _CACHE["nc"]

    in_maps = [
        _prep_core_inputs(inputs, c // HG, c % HG, NP) for c in range(NCORES)
    ]

    res = run_bass_kernel_spmd(nc, in_maps, core_ids=list(range(NCORES)))
    results = res.results

    bp = inputs["bp"].astype(np.float32)
    out = np.empty((B, N, OUT_DIM), np.float32)
    for b in range(B):
        acc = results[b * HG]["outT"].astype(np.float32)
        for g in range(1, HG):
            acc = acc + results[b * HG + g]["outT"].astype(np.float32)
        perm, nv = _perm_np(inputs["mask"][b, :, 0])
        out[b] = bp[None, :]
        out[b, perm[:nv]] = acc.T[:nv] + bp[None, :]
    return out
